# revision 1
# baseline (speedup 1.0000x reference)
"""Trainium2 Bass kernel for the GaussianRenderer problem.

Contract: kernel(data, opacity) -> img
  data:    (32, 512, 8) float32
  opacity: (512, 1)     float32
  returns  (32, 3, 64, 64) float32

Sharding: data-parallel over batch B=32 across 8 NeuronCores (4 images
per core); no collectives.

Per-core algorithm (all compute on device):
  sigma[n, p] is a rank-6 bilinear form: sigma = F[n, :6] @ G[:6, p]
  where G rows are the pixel-coordinate monomials [1, x, y, x^2, y^2, xy]
  with x, y integer in [-32, 31] (exactly representable in fp16). F is
  derived per gaussian on-device (tanh/sigmoid/sin on ScalarE, arithmetic
  on VectorE), split into fp16 hi/mid/lo parts and stacked K=18 so a
  single fp16 TensorE matmul per [128 gaussians x 512 pixels] tile yields
  fp32-accurate (negated) sigma. alpha = Exp(sigma_neg) runs on ScalarE
  reading PSUM directly, writing fp16 to SBUF. Blending is a second
  TensorE matmul contracting the 128-gaussian partition dim with
  opacity-scaled colors split hi/lo (lo placed at partitions 32-34 so the
  final combine is a legal aligned VectorE add), accumulated over the 4
  gaussian tiles in PSUM, then DMA'd out.
"""

import numpy as np

import concourse.bacc as bacc
import concourse.mybir as mybir
import concourse.tile as tile
from concourse import bass_utils
from concourse._compat import get_trn_type
from concourse.alu_op_type import AluOpType

F32 = mybir.dt.float32
F16 = mybir.dt.float16
AF = mybir.ActivationFunctionType

N_CORES = 8
B = 32
B_CORE = B // N_CORES  # 4 images per core
N = 512                # gaussians
NG = B_CORE * N        # gaussians handled per core
NT = 16                # gaussian tiles of 128 per core (4 img * 4 ntiles)
HW = 4096              # pixels per image (64 x 64)
PI = float(np.pi)

# pixel segments per (img, ntile): sized so the sigma PSUM tile (3 banks)
# double-buffers alongside the blend accumulator within the 8 PSUM banks.
SEGS = [(0, 1536), (1536, 1536), (3072, 1024)]


def host_constants():
    """G2 [18, 4096] fp16 (3 stacked copies of the monomial rows, for the
    hi/mid/lo K-stacking) + fp16 identity for the PE transpose."""
    xs = np.arange(64, dtype=np.float64) - 32.0
    Xg, Yg = np.meshgrid(xs, xs)  # [h, w]; row-major pixels p = h*64 + w
    G = np.stack(
        [np.ones_like(Xg), Xg, Yg, Xg * Xg, Yg * Yg, Xg * Yg], 0
    ).reshape(6, HW)
    G2 = np.concatenate([G, G, G], 0).astype(np.float16)  # [18, 4096]
    ident = np.eye(128, dtype=np.float16)
    return G2, ident


def build_program():
    nc = bacc.Bacc(get_trn_type() or "TRN2", target_bir_lowering=False, debug=False)
    d_data = nc.dram_tensor("data", (NG, 8), F32, kind="ExternalInput")
    d_opac = nc.dram_tensor("opacity", (N, 1), F32, kind="ExternalInput")
    d_g2 = nc.dram_tensor("gconst", (18, HW), F16, kind="ExternalInput")
    d_id = nc.dram_tensor("ident", (128, 128), F16, kind="ExternalInput")
    d_img = nc.dram_tensor("img", (B_CORE, 3, 64, 64), F32, kind="ExternalOutput")

    with tile.TileContext(nc) as tc:
        with (
            tc.tile_pool(name="const", bufs=1) as constp,
            tc.tile_pool(name="prep", bufs=1) as prep,
            tc.tile_pool(name="alpha", bufs=2) as alphap,
            tc.tile_pool(name="outp", bufs=4) as outp,
        ):
            # ---- constants + inputs to SBUF ----
            g2 = constp.tile([18, HW], F16, tag="g2")
            nc.sync.dma_start(g2[:], d_g2[:])
            idt = constp.tile([128, 128], F16, tag="idt")
            nc.sync.dma_start(idt[:], d_id[:])
            d8 = constp.tile([128, 128], F32, tag="d8")  # [p, t*8+k]
            nc.sync.dma_start(
                d8.rearrange("p (t k) -> p t k", k=8),
                d_data.rearrange("(t p) k -> p t k", p=128),
            )
            opac = constp.tile([128, 4], F32, tag="opac")  # [p, ntile]
            nc.sync.dma_start(
                opac[:], d_opac.rearrange("(nt p) one -> p (nt one)", p=128)
            )

            d8v = d8.rearrange("p (t k) -> p t k", k=8)

            def field(k):  # [128, 16] strided view of input field k
                return d8v[:, :, k]

            def t16(tag):
                return prep.tile([128, 16], F32, tag=tag, name=tag)

            # ---- per-gaussian preprocessing ([128, 16] fp32 tiles) ----
            # theta = 2*pi*sigmoid(d4). Build sin/cos(2*theta) from
            # half-angle pieces so ACT Sin only sees args in (-pi/2, pi):
            #   s1 = sin(pi*sg), nc1 = sin(pi*sg - pi/2) = -cos(pi*sg)
            #   sin(theta) = -2*s1*nc1, cos(theta) = 1 - 2*s1^2, then
            #   double-angle once more for sin/cos(2*theta).
            sg = t16("sg")
            nc.scalar.activation(sg[:], field(4), AF.Sigmoid)
            a1 = t16("a1")
            nc.vector.tensor_scalar_mul(a1[:], sg[:], PI)
            s1 = t16("s1")
            nc.scalar.activation(s1[:], a1[:], AF.Sin)
            a2 = t16("a2")
            nc.vector.tensor_scalar(
                a2[:], sg[:], PI, -PI / 2, AluOpType.mult, AluOpType.add
            )
            nc1 = t16("nc1")
            nc.scalar.activation(nc1[:], a2[:], AF.Sin)
            sth = t16("sth")  # sin(theta)
            nc.vector.scalar_tensor_tensor(
                sth[:], s1[:], -2.0, nc1[:], AluOpType.mult, AluOpType.mult
            )
            cth = t16("cth")  # cos(theta) = 1 - 2*s1^2
            nc.vector.tensor_tensor(cth[:], s1[:], s1[:], AluOpType.mult)
            nc.vector.tensor_scalar(
                cth[:], cth[:], -2.0, 1.0, AluOpType.mult, AluOpType.add
            )
            s2t = t16("s2t")  # sin(2*theta) = 2*sin(theta)*cos(theta)
            nc.vector.scalar_tensor_tensor(
                s2t[:], sth[:], 2.0, cth[:], AluOpType.mult, AluOpType.mult
            )
            c2t = t16("c2t")  # cos(2*theta) = 1 - 2*sin(theta)^2
            nc.vector.tensor_tensor(c2t[:], sth[:], sth[:], AluOpType.mult)
            nc.vector.tensor_scalar(
                c2t[:], c2t[:], -2.0, 1.0, AluOpType.mult, AluOpType.add
            )

            # centers (global shift -32): ex = 32*tanh(d0) - 0.5
            ex = t16("ex")
            nc.scalar.activation(ex[:], field(0), AF.Tanh)
            nc.vector.tensor_scalar(
                ex[:], ex[:], 32.0, -0.5, AluOpType.mult, AluOpType.add
            )
            ey = t16("ey")
            nc.scalar.activation(ey[:], field(1), AF.Tanh)
            nc.vector.tensor_scalar(
                ey[:], ey[:], 32.0, -0.5, AluOpType.mult, AluOpType.add
            )

            # scales: h0 = 0.5*(|d2|+0.3)^2, h1 = 0.5*(|d3|+0.3)^2
            s0 = t16("s0")
            nc.scalar.activation(s0[:], field(2), AF.Abs)
            nc.vector.tensor_scalar_add(s0[:], s0[:], 0.3)
            s1s = t16("s1s")
            nc.scalar.activation(s1s[:], field(3), AF.Abs)
            nc.vector.tensor_scalar_add(s1s[:], s1s[:], 0.3)
            h0 = t16("h0")
            nc.vector.tensor_tensor(h0[:], s0[:], s0[:], AluOpType.mult)
            nc.vector.tensor_scalar_mul(h0[:], h0[:], 0.5)
            h1 = t16("h1")
            nc.vector.tensor_tensor(h1[:], s1s[:], s1s[:], AluOpType.mult)
            nc.vector.tensor_scalar_mul(h1[:], h1[:], 0.5)

            sum5 = t16("sum5")  # 0.5*(s0^2+s1^2)
            nc.vector.tensor_tensor(sum5[:], h0[:], h1[:], AluOpType.add)
            dif5 = t16("dif5")  # 0.5*(s0^2-s1^2)
            nc.vector.tensor_tensor(dif5[:], h0[:], h1[:], AluOpType.subtract)

            # covariance entries
            dc = t16("dc")
            nc.vector.tensor_tensor(dc[:], dif5[:], c2t[:], AluOpType.mult)
            cov_a = t16("cov_a")
            nc.vector.tensor_tensor(cov_a[:], sum5[:], dc[:], AluOpType.add)
            cov_c = t16("cov_c")
            nc.vector.tensor_tensor(cov_c[:], sum5[:], dc[:], AluOpType.subtract)
            cov_b = t16("cov_b")
            nc.vector.tensor_tensor(cov_b[:], dif5[:], s2t[:], AluOpType.mult)

            det = t16("det")
            nc.vector.tensor_tensor(det[:], cov_a[:], cov_c[:], AluOpType.mult)
            bb = t16("bb")
            nc.vector.tensor_tensor(bb[:], cov_b[:], cov_b[:], AluOpType.mult)
            nc.vector.tensor_tensor(det[:], det[:], bb[:], AluOpType.subtract)

            # conic: ca = cov_c/det, cc = cov_a/det, cbn = cov_b/det (= -con_b)
            inv = t16("inv")
            nc.vector.reciprocal(inv[:], det[:])
            ca = t16("ca")
            nc.vector.tensor_tensor(ca[:], cov_c[:], inv[:], AluOpType.mult)
            cc = t16("cc")
            nc.vector.tensor_tensor(cc[:], cov_a[:], inv[:], AluOpType.mult)
            cbn = t16("cbn")
            nc.vector.tensor_tensor(cbn[:], cov_b[:], inv[:], AluOpType.mult)

            # ---- F rows (negated for exp), written into Fc [128, 96] ----
            Fc = prep.tile([128, 96], F32, tag="Fc")
            Fv = Fc.rearrange("p (t k) -> p t k", k=6)

            exq = t16("exq")
            nc.vector.tensor_tensor(exq[:], ex[:], ex[:], AluOpType.mult)
            eyq = t16("eyq")
            nc.vector.tensor_tensor(eyq[:], ey[:], ey[:], AluOpType.mult)
            exey = t16("exey")
            nc.vector.tensor_tensor(exey[:], ex[:], ey[:], AluOpType.mult)

            # f0 = -0.5*ca*exq - 0.5*cc*eyq + cbn*exey
            t_a = t16("t_a")
            nc.vector.tensor_tensor(t_a[:], ca[:], exq[:], AluOpType.mult)
            t_b = t16("t_b")
            nc.vector.tensor_tensor(t_b[:], cc[:], eyq[:], AluOpType.mult)
            nc.vector.tensor_tensor(t_a[:], t_a[:], t_b[:], AluOpType.add)
            nc.vector.tensor_scalar_mul(t_a[:], t_a[:], -0.5)
            t_c = t16("t_c")
            nc.vector.tensor_tensor(t_c[:], cbn[:], exey[:], AluOpType.mult)
            nc.vector.tensor_tensor(Fv[:, :, 0], t_a[:], t_c[:], AluOpType.add)

            # f_x = ca*ex - cbn*ey ; f_y = cc*ey - cbn*ex
            nc.vector.tensor_tensor(t_a[:], ca[:], ex[:], AluOpType.mult)
            nc.vector.tensor_tensor(t_b[:], cbn[:], ey[:], AluOpType.mult)
            nc.vector.tensor_tensor(Fv[:, :, 1], t_a[:], t_b[:], AluOpType.subtract)
            nc.vector.tensor_tensor(t_a[:], cc[:], ey[:], AluOpType.mult)
            nc.vector.tensor_tensor(t_b[:], cbn[:], ex[:], AluOpType.mult)
            nc.vector.tensor_tensor(Fv[:, :, 2], t_a[:], t_b[:], AluOpType.subtract)

            # f_x2 = -0.5*ca ; f_y2 = -0.5*cc ; f_xy = +cbn
            nc.vector.tensor_scalar_mul(Fv[:, :, 3], ca[:], -0.5)
            nc.vector.tensor_scalar_mul(Fv[:, :, 4], cc[:], -0.5)
            nc.vector.tensor_scalar_mul(Fv[:, :, 5], cbn[:], 1.0)

            # ---- split F into fp16 hi/mid/lo, interleaved [128, 16*18] ----
            fall = prep.tile([128, NT * 18], F16, tag="fall")
            fv = fall.rearrange("p (t s) -> p t s", s=18)
            Fc6 = Fc.rearrange("p (t k) -> p t k", k=6)
            nc.vector.tensor_copy(fv[:, :, 0:6], Fc6[:, :, :])
            r1 = prep.tile([128, 96], F32, tag="r1")
            r16 = r1.rearrange("p (t k) -> p t k", k=6)
            nc.vector.tensor_tensor(
                r16[:, :, :], Fc6[:, :, :], fv[:, :, 0:6], AluOpType.subtract
            )
            nc.vector.tensor_copy(fv[:, :, 6:12], r16[:, :, :])
            r2 = prep.tile([128, 96], F32, tag="r2")
            r26 = r2.rearrange("p (t k) -> p t k", k=6)
            nc.vector.tensor_tensor(
                r26[:, :, :], r16[:, :, :], fv[:, :, 6:12], AluOpType.subtract
            )
            nc.vector.tensor_copy(fv[:, :, 12:18], r26[:, :, :])

            # ---- per-tile transpose: [128, 18] -> psum [18, 128] -> f2 ----
            f2 = constp.tile([18, NT * 128], F16, tag="f2")  # matmul weights
            with tc.tile_pool(name="prepps", bufs=2, space="PSUM") as prepps:
                for t in range(NT):
                    tp = prepps.tile([18, 128], F16, tag="tp", name=f"tp{t}")
                    nc.tensor.transpose(tp[:], fall[:, t * 18 : (t + 1) * 18], idt[:])
                    nc.vector.tensor_copy(f2[:, t * 128 : (t + 1) * 128], tp[:])

            # ---- colors * opacity, split hi/lo -> c2 [128, 16*35] fp16 ----
            # hi at cols t*35+{0,1,2}, lo at t*35+{32,33,34}: the blend
            # matmul then lands lo rows at PSUM partitions 32-34, which the
            # 32-aligned VectorE combine can read.
            opac_b = opac[:].unsqueeze(1).broadcast_to([128, 4, 4])
            cP = prep.tile([128, 48], F32, tag="cP")
            cPv = cP.rearrange("p (t k) -> p t k", k=3)
            cP4 = cP.rearrange("p (i n k) -> p i n k", n=4, k=3)
            d84 = d8.rearrange("p (i n k) -> p i n k", n=4, k=8)
            for k in range(3):
                nc.vector.tensor_tensor(
                    cP4[:, :, :, k], d84[:, :, :, 5 + k], opac_b, AluOpType.mult
                )
            c2 = constp.tile([128, NT * 35], F16, tag="c2")
            nc.vector.memset(c2[:], 0.0)
            c2v = c2.rearrange("p (t s) -> p t s", s=35)
            nc.vector.tensor_copy(c2v[:, :, 0:3], cPv[:, :, :])
            chi32 = prep.tile([128, 48], F32, tag="chi32")
            nc.vector.tensor_copy(
                chi32.rearrange("p (t k) -> p t k", k=3)[:, :, :], c2v[:, :, 0:3]
            )
            rlo = prep.tile([128, 48], F32, tag="rlo")
            nc.vector.tensor_tensor(rlo[:], cP[:], chi32[:], AluOpType.subtract)
            nc.vector.tensor_copy(
                c2v[:, :, 32:35], rlo.rearrange("p (t k) -> p t k", k=3)[:, :, :]
            )

            # ---- main loop ----
            with (
                tc.tile_pool(name="sigps", bufs=2, space="PSUM") as sigps,
                tc.tile_pool(name="blps", bufs=2, space="PSUM") as blps,
            ):
                for img in range(B_CORE):
                    al = alphap.tile([128, 4 * HW], F16, tag="al", name=f"al{img}")
                    for nt in range(4):
                        t = img * 4 + nt
                        w = f2[:, t * 128 : (t + 1) * 128]
                        for off, ln in SEGS:
                            sps = sigps.tile(
                                [128, ln], F32, tag="sig", name=f"sig{img}_{nt}_{off}"
                            )
                            for q in range(0, ln, 512):
                                nc.tensor.matmul(
                                    sps[:, q : q + 512],
                                    w,
                                    g2[:, off + q : off + q + 512],
                                    start=True,
                                    stop=True,
                                )
                            nc.scalar.activation(
                                al[:, nt * HW + off : nt * HW + off + ln],
                                sps[:],
                                AF.Exp,
                            )
                    for ch in range(8):
                        bps = blps.tile(
                            [35, 512], F32, tag="bl", name=f"bl{img}_{ch}"
                        )
                        for nt in range(4):
                            t = img * 4 + nt
                            nc.tensor.matmul(
                                bps[:],
                                c2[:, t * 35 : t * 35 + 35],
                                al[:, nt * HW + ch * 512 : nt * HW + ch * 512 + 512],
                                start=(nt == 0),
                                stop=(nt == 3),
                            )
                        ot = outp.tile([3, 512], F32, tag="ot", name=f"ot{img}_{ch}")
                        nc.vector.tensor_copy(ot[:], bps[32:35, :])
                        nc.vector.tensor_tensor(
                            ot[:], bps[0:3, :], ot[:], AluOpType.add
                        )
                        nc.sync.dma_start(
                            d_img[img, :, ch * 8 : (ch + 1) * 8, :].rearrange(
                                "c h w -> c (h w)"
                            ),
                            ot[:],
                        )

    nc.compile()
    return nc


_NC_CACHE = None


def _get_program():
    global _NC_CACHE
    if _NC_CACHE is None:
        _NC_CACHE = build_program()
    return _NC_CACHE


def make_in_maps(data, opacity):
    data = np.ascontiguousarray(np.asarray(data, dtype=np.float32))
    opacity = np.ascontiguousarray(np.asarray(opacity, dtype=np.float32))
    G2, ident = host_constants()
    in_maps = []
    for c in range(N_CORES):
        in_maps.append(
            {
                "data": np.ascontiguousarray(
                    data[c * B_CORE : (c + 1) * B_CORE].reshape(NG, 8)
                ),
                "opacity": opacity,
                "gconst": G2,
                "ident": ident,
            }
        )
    return in_maps


def kernel(data, opacity):
    nc = _get_program()
    in_maps = make_in_maps(data, opacity)
    res = bass_utils.run_bass_kernel_spmd(nc, in_maps, core_ids=list(range(N_CORES)))
    out = np.concatenate(
        [res.results[c]["img"] for c in range(N_CORES)], axis=0
    ).astype(np.float32)
    return out
